# revision 1
# baseline (speedup 1.0000x reference)
"""Bahdanau additive attention Trainium2 Bass kernel.

Reference (per batch b):
    U = key @ W_encoder.T                  # [S, A]
    V = q @ W_decoder.T                    # [A]
    score = tanh(U + V) @ v[0]             # [S]
    w = softmax(score)                     # [S]
    context = w @ key                      # [KD]

Sharding: data-parallel over batch across 8 NeuronCores (4 batches/core),
weights replicated.  All heavy matmuls run in fp32r (fp32 with 11-bit
mantissa, full PE rate at free-dim >= 256, ~2e-4 relative error).

Per-core pipeline, per batch, per s-chunk of 512:
  1. SWDGE DMA-cast key chunk fp32->fp32r into SBUF (native [s,k] layout).
  2. keyT [k, s] tiles via normal-mode PE matmuls out = key_block.T @ I
     (contraction over the s partition dim; measured much faster than the
     dedicated transpose mode for this 128x128 fp32r case).
  3. U^T tiles [a=128, s=512] = WeT.T @ keyT accumulated over k in PSUM.
  4. ACT: tanh(U^T + V[a]) from PSUM (V as per-partition bias), fp32r out.
  5. score [1, 512] = v.T @ tanh-tiles accumulated over a-tiles on PE.
  6. ACT: e = exp(score) (no max subtraction needed: |score| <= sum|v| ~ 26,
     well inside fp32 range), accum_out gives the chunk's sum(e).
  7. PE-transpose e-row into an e-column tile [128, 4].
  8. context PSUM [1, 1024] += e-col.T @ key-native, accumulated across all
     chunks; key chunk is then dead (single pass over key).
Batch epilogue: Z = sum of chunk sums, context * (1/Z) on DVE, DMA out.
"""
import sys
sys.path.insert(0, "/opt/trn_rl_repo")

from contextlib import ExitStack

import numpy as np

import concourse.bass as bass
import concourse.tile as tile
from concourse import bacc, masks, mybir

dt = mybir.dt
AF = mybir.ActivationFunctionType

# Full problem shape
B, S, KD, QD, AD = 32, 2048, 1024, 1024, 1024
N_CORES = 8
BS = B // N_CORES          # batches per core
SC = 512                   # s-chunk (columns per U matmul)


def build_kernel(nc, bs=BS, s=S, kd=KD, qd=QD, ad=AD, reps=1, dyn_reps=0):
    """Emit the per-core kernel into `nc` (a bacc.Bacc).

    reps>1 statically unrolls the whole pipeline (timing amplification);
    dyn_reps>0 instead wraps it in a hardware For_i loop.
    """
    f32, f32r = dt.float32, dt.float32r
    nsc = s // SC            # s-chunks per batch
    nkt = kd // 128          # k-tiles
    nat = ad // 128          # a-tiles
    nqt = qd // 128          # q-tiles
    assert s % SC == 0 and kd % 128 == 0 and ad % 128 == 0 and qd % 128 == 0

    key_d = nc.dram_tensor("key", [bs, s, kd], f32, kind="ExternalInput").ap()
    q_d = nc.dram_tensor("q", [bs, qd], f32, kind="ExternalInput").ap()
    we_d = nc.dram_tensor("W_encoder", [ad, kd], f32, kind="ExternalInput").ap()
    wd_d = nc.dram_tensor("W_decoder", [ad, qd], f32, kind="ExternalInput").ap()
    v_d = nc.dram_tensor("v", [1, ad], f32, kind="ExternalInput").ap()
    out_d = nc.dram_tensor("out", [bs, kd], f32, kind="ExternalOutput").ap()

    with tile.TileContext(nc) as tc, ExitStack() as ctx:
        const = ctx.enter_context(tc.tile_pool(name="const", bufs=1))

        ident_f = const.tile([128, 128], f32, name="ident_f")
        masks.make_identity(nc, ident_f[:])
        ident_r = const.tile([128, 128], f32r, name="ident_r")
        nc.vector.tensor_copy(ident_r[:], ident_f[:])
        one_f = const.tile([1, 1], f32, name="one_f")
        nc.gpsimd.memset(one_f[:], 1.0)
        one_r = const.tile([1, 1], f32r, name="one_r")
        nc.vector.tensor_copy(one_r[:], one_f[:])

        # WeT[k, a] tiles (fp32r), one [128, ad] tile per k-tile.
        wet = [const.tile([128, ad], f32r, name=f"wet{t}") for t in range(nkt)]
        # V bias [a-tile][128, bs] fp32 and v columns [128, nat] fp32r.
        vbias = [const.tile([128, bs], f32, name=f"vbias{m}") for m in range(nat)]
        vcols = const.tile([128, nat], f32r, name="vcols")

        # ---------------- pools ----------------
        kpool = ctx.enter_context(tc.tile_pool(name="knat", bufs=3))
        ktpool = ctx.enter_context(tc.tile_pool(name="keyT", bufs=2))
        thpool = ctx.enter_context(tc.tile_pool(name="tanh", bufs=2))
        spool = ctx.enter_context(tc.tile_pool(name="small", bufs=2))
        pp_t = ctx.enter_context(tc.tile_pool(name="pp_t", bufs=2, space="PSUM"))
        pp_u = ctx.enter_context(tc.tile_pool(name="pp_u", bufs=2, space="PSUM"))
        pp_s = ctx.enter_context(tc.tile_pool(name="pp_s", bufs=1, space="PSUM"))
        pp_c = ctx.enter_context(tc.tile_pool(name="pp_c", bufs=1, space="PSUM"))

        def load_tp_chunk(uid, b, c):
            """DMA-cast one key chunk and emit its transpose matmuls.
            Returns (knat3, kts)."""
            knat = kpool.tile([128, 4 * kd], f32r, name=f"knat{uid}",
                              tag="knat")
            knat3 = knat[:].rearrange("p (t k) -> p t k", k=kd)
            nc.gpsimd.dma_start(
                knat3,
                key_d[b, c * SC:(c + 1) * SC, :]
                .rearrange("(t p) k -> p t k", p=128))
            # keyT tiles via normal-mode PE "transpose": out = block.T @ I
            # (contraction over the s partition dim) — much faster than
            # transpose mode for 128x128 fp32r.
            kts = []
            for t in range(nkt):
                pst = pp_t.tile([128, SC], f32,
                                name=f"pst{uid}t{t}", tag="pst")
                for sp in range(4):
                    nc.tensor.matmul(
                        pst[:, sp * 128:(sp + 1) * 128],
                        knat3[:, sp, t * 128:(t + 1) * 128],
                        ident_r[:], start=True, stop=True)
                kt = ktpool.tile([128, SC], f32r,
                                 name=f"kt{uid}t{t}", tag=f"kt{t}")
                nc.vector.tensor_copy(kt[:], pst[:])
                kts.append(kt)
            return knat3, kts

        # Hoist chunk (b=0, c=0): its key DMA goes out on the SWDGE queue
        # BEFORE the 16MB of weight DMAs, and its transpose matmuls lead
        # the PE program so the PE has work while the weights stream in.
        pre = None
        if not dyn_reps and reps == 1:
            pre = load_tp_chunk("pre", 0, 0)

        # ---------------- weight prep (once per core) ----------------
        # Weights DMA-cast to fp32r; transposed in 2 half-loads (4 row
        # tiles resident at a time) to keep SBUF under budget now that the
        # prep overlaps the main-loop pools.  PSUM is borrowed from the
        # main-loop pools.
        with tc.tile_pool(name="wprep", bufs=1) as wprep:
            w_nat = [wprep.tile([128, kd], f32r, name=f"wnat{m}",
                                tag=f"wn{m}") for m in range(4)]

            def build_transposed(dst_tiles, src_d):
                for half in range(0, nat, 4):
                    n = min(4, nat - half)
                    for j in range(n):
                        nc.gpsimd.dma_start(
                            w_nat[j][:],
                            src_d[(half + j) * 128:(half + j + 1) * 128, :])
                    for t in range(nkt):
                        ps = pp_t.tile([128, 512], f32,
                                       name=f"wps{id(dst_tiles)}_{t}_{half}",
                                       tag="pst")
                        for j in range(n):
                            nc.tensor.matmul(
                                ps[:, j * 128:(j + 1) * 128],
                                w_nat[j][:, t * 128:(t + 1) * 128],
                                ident_r[:], start=True, stop=True)
                        nc.vector.tensor_copy(
                            dst_tiles[t][:, half * 128:(half + n) * 128],
                            ps[:, :n * 128])

            # --- WeT (into persistent const tiles) ---
            build_transposed(wet, we_d)

            # --- qT ---
            qn = wprep.tile([bs, qd], f32, name="qn")
            nc.sync.dma_start(qn[:], q_d)
            psq = pp_s.tile([128, nqt * bs], f32, name="psq", tag="pse")
            for t in range(nqt):
                nc.tensor.matmul(psq[:, t * bs:(t + 1) * bs],
                                 qn[:, t * 128:(t + 1) * 128],
                                 ident_f[:bs, :bs], is_transpose=True)
            qt = wprep.tile([128, nqt * bs], f32r, name="qt")
            nc.vector.tensor_copy(qt[:], psq[:])

            # --- V = q @ Wd.T, streaming Wd one q-column-tile at a time.
            # All a-tiles accumulate side by side in one PSUM bank.
            psv_all = pp_s.tile([128, nat * bs], f32, name="psv_all",
                                tag="pss")
            for t in range(nqt):
                wdc = wprep.tile([128, nat * 128], f32r, name=f"wdc{t}",
                                 tag="wdc", bufs=2)
                wdc3 = wdc[:].rearrange("p (j q) -> p j q", q=128)
                nc.gpsimd.dma_start(
                    wdc3,
                    wd_d[:, t * 128:(t + 1) * 128]
                    .rearrange("(j p) q -> p j q", p=128))
                wdt_t = wprep.tile([128, ad], f32r, name=f"wdt{t}",
                                   tag="wdt", bufs=2)
                for j in range(nat):
                    ps = pp_t.tile([128, 512], f32, name=f"wdps{t}_{j}",
                                   tag="pst", bufs=2)
                    nc.tensor.matmul(ps[:, :128], wdc3[:, j, :],
                                     ident_r[:], start=True, stop=True)
                    nc.vector.tensor_copy(wdt_t[:, j * 128:(j + 1) * 128],
                                          ps[:, :128])
                for m in range(nat):
                    # one accumulation group spans the whole bank: the
                    # t==0/m==0 start clears the bank, later m slices
                    # overwrite-where-clear then accumulate over t
                    nc.tensor.matmul(
                        psv_all[:, m * bs:(m + 1) * bs],
                        wdt_t[:, m * 128:(m + 1) * 128],
                        qt[:, t * bs:(t + 1) * bs],
                        start=(t == 0 and m == 0),
                        stop=(t == nqt - 1 and m == nat - 1),
                        skip_group_check=True)
            for m in range(nat):
                nc.vector.tensor_copy(vbias[m][:],
                                      psv_all[:, m * bs:(m + 1) * bs])

            # --- v columns ---
            vrow = wprep.tile([1, ad], f32, name="vrow")
            nc.sync.dma_start(vrow[:], v_d)
            psvc = pp_s.tile([128, nat], f32, name="psvc", tag="pss")
            for m in range(nat):
                nc.tensor.matmul(psvc[:, m:m + 1],
                                 vrow[:, m * 128:(m + 1) * 128],
                                 one_f[:], is_transpose=True)
            nc.vector.tensor_copy(vcols[:], psvc[:])

        # ---------------- main streaming loop ----------------
        nkh = kd // 512  # context free-dim chunks

        def emit_body(rep):
            for b in range(bs):
                tagb = f"r{rep}b{b}"
                zparts = spool.tile([1, nsc], f32, name=f"zp{tagb}",
                                    tag="zparts")
                ctx_ps = [pp_c.tile([1, 512], f32, name=f"ctx{tagb}_{h}",
                                    tag=f"ctx{h}") for h in range(nkh)]
                def emit_tail(c, erow, knat3):
                    # 7. e-row -> e-columns [128, 4] (fp32 transpose-mode;
                    # a normal fp32 K=1 matmul here measured ~2us each, and
                    # an SBUF->SBUF scatter DMA gave wrong results on HW)
                    pse = pp_s.tile([128, 4], f32, name=f"pse{tagb}c{c}",
                                    tag="pse")
                    for sp in range(4):
                        nc.tensor.matmul(pse[:, sp:sp + 1],
                                         erow[:, sp * 128:(sp + 1) * 128],
                                         one_f[:], is_transpose=True)
                    ecol = spool.tile([128, 4], f32r, name=f"ec{tagb}c{c}",
                                      tag="ecol")
                    nc.vector.tensor_copy(ecol[:], pse[:])

                    # 8. context accumulation (contract over s)
                    for sp in range(4):
                        for h in range(nkh):
                            nc.tensor.matmul(
                                ctx_ps[h][:], ecol[:, sp:sp + 1],
                                knat3[:, sp, h * 512:(h + 1) * 512],
                                start=(c == 0 and sp == 0),
                                stop=(c == nsc - 1 and sp == 3))

                pending = None
                for c in range(nsc):
                    # 1+2. key chunk load + keyT transposes (the very first
                    # chunk may have been hoisted ahead of weight prep)
                    if pre is not None and rep == 0 and b == 0 and c == 0:
                        knat3, kts = pre
                    else:
                        knat3, kts = load_tp_chunk(f"{tagb}c{c}", b, c)

                    # 3+4. U^T a-tiles, tanh(U+V) on ACT
                    ths = []
                    for m in range(nat):
                        psu = pp_u.tile([128, SC], f32,
                                        name=f"psu{tagb}c{c}m{m}", tag="psu")
                        for t in range(nkt):
                            nc.tensor.matmul(
                                psu[:], wet[t][:, m * 128:(m + 1) * 128],
                                kts[t][:],
                                start=(t == 0), stop=(t == nkt - 1))
                        th = thpool.tile([128, SC], f32r,
                                         name=f"th{tagb}c{c}m{m}", tag=f"th{m}")
                        nc.scalar.activation(th[:], psu[:], AF.Tanh,
                                             bias=vbias[m][:, b:b + 1])
                        ths.append(th)

                    # 5. score row
                    pss = pp_s.tile([1, SC], f32, name=f"pss{tagb}c{c}",
                                    tag="pss")
                    for m in range(nat):
                        nc.tensor.matmul(pss[:], vcols[:, m:m + 1], ths[m][:],
                                         start=(m == 0), stop=(m == nat - 1))

                    # 6. e = exp(score); chunk sum via accum_out
                    erow = spool.tile([1, SC], f32, name=f"erow{tagb}c{c}",
                                      tag="erow")
                    nc.scalar.activation(erow[:], pss[:], AF.Exp,
                                         accum_out=zparts[:, c:c + 1])

                    # 7+8 for the PREVIOUS chunk: deferred one chunk so the
                    # PE never stalls at the e-column matmuls waiting for
                    # ACT's exp — by now exp(c-1) has long completed.
                    if pending is not None:
                        emit_tail(*pending)
                    pending = (c, erow, knat3)
                emit_tail(*pending)

                # batch epilogue: normalize and store
                z = spool.tile([1, 1], f32, name=f"z{tagb}", tag="z")
                nc.vector.reduce_sum(z[:], zparts[:], axis=mybir.AxisListType.X)
                rz = spool.tile([1, 1], f32, name=f"rz{tagb}", tag="rz")
                nc.vector.reciprocal(rz[:], z[:])
                cout = spool.tile([1, kd], f32, name=f"cout{tagb}", tag="cout")
                for h in range(nkh):
                    nc.vector.tensor_scalar_mul(cout[:, h * 512:(h + 1) * 512],
                                                ctx_ps[h][:], rz[:])
                nc.sync.dma_start(out_d[b:b + 1, :], cout[:])

        if dyn_reps:
            with tc.For_i(0, dyn_reps, 1):
                emit_body(0)
        else:
            for rep in range(reps):
                emit_body(rep)
    return nc


_CACHE = {}


def _get_compiled(cfg):
    if cfg not in _CACHE:
        nc = bacc.Bacc("TRN2", target_bir_lowering=False, debug=False)
        build_kernel(nc, *cfg)
        nc.compile()
        _CACHE[cfg] = nc
    return _CACHE[cfg]


def kernel(**inputs):
    from concourse.bass_utils import run_bass_kernel_spmd

    key = np.asarray(inputs["key"], dtype=np.float32)
    q = np.asarray(inputs["q"], dtype=np.float32)
    we = np.asarray(inputs["W_encoder"], dtype=np.float32)
    wd = np.asarray(inputs["W_decoder"], dtype=np.float32)
    v = np.asarray(inputs["v"], dtype=np.float32)

    nc = _get_compiled((BS, S, KD, QD, AD, 1))
    in_maps = []
    for cidx in range(N_CORES):
        sl = slice(cidx * BS, (cidx + 1) * BS)
        in_maps.append({
            "key": key[sl], "q": q[sl],
            "W_encoder": we, "W_decoder": wd, "v": v,
        })
    res = run_bass_kernel_spmd(nc, in_maps, list(range(N_CORES))).results
    return np.concatenate([r["out"] for r in res], axis=0)


if __name__ == "__main__":
    # quick smoke: random small check against numpy on this module's math
    pass



# revision 2
# speedup vs baseline: 1.5211x; 1.5211x over previous
"""Bahdanau additive attention Trainium2 Bass kernel (fp8 DoubleRow U).

Reference (per batch b):
    U = key @ W_encoder.T                  # [S, A]
    V = q @ W_decoder.T                    # [A]
    score = tanh(U + V) @ v[0]             # [S]
    w = softmax(score)                     # [S]
    context = w @ key                      # [KD]

Sharding: data-parallel over batch across 8 NeuronCores (4 batches/core),
weights replicated.

Precision plan (validated numerically against the fp32 reference on the
actual seed-0 inputs, rel-err ~1.1e-2 vs 2e-2 budget):
  - The dominant U matmul runs in fp8e4 (e4m3) with DoubleRow perf mode:
    each PE instruction contracts TWO 128-deep k-tiles (0.5 cyc/row).
  - key is DMA-cast fp32->bf16 into SBUF (native [s,k] layout); the
    context matmul and the PE transposes read bf16 (context in fp8 would
    blow the error budget).
  - keyT tiles are produced by normal-mode PE matmuls against a bf16
    identity; the PSUM->SBUF copy casts to fp8 for the U matmul.
  - tanh(U+V) -> bf16, score matmul bf16, softmax/context f32 epilogue.

Per-core pipeline, per batch, per s-chunk of 512:
  1. SWDGE DMA-cast key chunk fp32->bf16 into SBUF (native [s,k] layout).
  2. keyT pair-tiles [k=128, 2, s=512] fp8 via PE identity matmuls + DVE
     PSUM->SBUF fp8 cast.
  3. U^T tiles [a=128, s=512] = 4 DoubleRow fp8 matmuls (2 k-tiles each)
     accumulated in PSUM.
  4. ACT: tanh(U^T + V[a]) from PSUM (V as per-partition bias), bf16 out.
  5. score [1, 512] = v.T @ tanh-tiles accumulated over a-tiles (bf16).
  6. ACT: e = exp(score) (|score| small, no max subtraction), accum_out
     gives the chunk's sum(e).
  7. PE-transpose e-row into an e-column tile [128, 4] (bf16).
  8. context PSUM [1, 1024] += e-col.T @ key-native (bf16), accumulated
     across all chunks.
Batch epilogue: Z = sum of chunk sums, context * (1/Z) on DVE, DMA out.
"""
import sys
sys.path.insert(0, "/opt/trn_rl_repo")

from contextlib import ExitStack

import numpy as np

import concourse.bass as bass
import concourse.tile as tile
from concourse import bacc, masks, mybir

dt = mybir.dt
AF = mybir.ActivationFunctionType
DR = mybir.MatmulPerfMode.DoubleRow

# Full problem shape
B, S, KD, QD, AD = 32, 2048, 1024, 1024, 1024
N_CORES = 8
BS = B // N_CORES          # batches per core
SC = 512                   # s-chunk (columns per U matmul)


def build_kernel(nc, bs=BS, s=S, kd=KD, qd=QD, ad=AD, reps=1, dyn_reps=0):
    """Emit the per-core kernel into `nc` (a bacc.Bacc)."""
    f32, f32r, bf16, f8 = dt.float32, dt.float32r, dt.bfloat16, dt.float8e4
    nsc = s // SC            # s-chunks per batch
    nkt = kd // 128          # k-tiles
    nkp = nkt // 2           # k-tile pairs (DoubleRow)
    nat = ad // 128          # a-tiles
    nqt = qd // 128          # q-tiles
    assert s % SC == 0 and kd % 256 == 0 and ad % 128 == 0 and qd % 128 == 0

    key_d = nc.dram_tensor("key", [bs, s, kd], f32, kind="ExternalInput").ap()
    q_d = nc.dram_tensor("q", [bs, qd], f32, kind="ExternalInput").ap()
    we_d = nc.dram_tensor("W_encoder", [ad, kd], f32, kind="ExternalInput").ap()
    wd_d = nc.dram_tensor("W_decoder", [ad, qd], f32, kind="ExternalInput").ap()
    v_d = nc.dram_tensor("v", [1, ad], f32, kind="ExternalInput").ap()
    out_d = nc.dram_tensor("out", [bs, kd], f32, kind="ExternalOutput").ap()

    with tile.TileContext(nc) as tc, ExitStack() as ctx:
        const = ctx.enter_context(tc.tile_pool(name="const", bufs=1))

        ident_f = const.tile([128, 128], f32, name="ident_f")
        masks.make_identity(nc, ident_f[:])
        ident_r = const.tile([128, 128], f32r, name="ident_r")
        nc.vector.tensor_copy(ident_r[:], ident_f[:])
        ident_b = const.tile([128, 128], bf16, name="ident_b")
        nc.vector.tensor_copy(ident_b[:], ident_f[:])
        one_f = const.tile([1, 1], f32, name="one_f")
        nc.gpsimd.memset(one_f[:], 1.0)

        # WeT pair tiles [k=128, 2, a=ad] fp8, one per k-tile pair.
        wet = [const.tile([128, 2, ad], f8, name=f"wet{t}") for t in range(nkp)]
        # V bias [a-tile][128, bs] fp32 and v columns [128, nat] bf16.
        vbias = [const.tile([128, bs], f32, name=f"vbias{m}") for m in range(nat)]
        vcols = const.tile([128, nat], bf16, name="vcols")

        # ---------------- pools ----------------
        kpool = ctx.enter_context(tc.tile_pool(name="knat", bufs=3))
        ktpool = ctx.enter_context(tc.tile_pool(name="keyT", bufs=2))
        thpool = ctx.enter_context(tc.tile_pool(name="tanh", bufs=2))
        spool = ctx.enter_context(tc.tile_pool(name="small", bufs=2))
        pp_t = ctx.enter_context(tc.tile_pool(name="pp_t", bufs=2, space="PSUM"))
        pp_u = ctx.enter_context(tc.tile_pool(name="pp_u", bufs=2, space="PSUM"))
        pp_s = ctx.enter_context(tc.tile_pool(name="pp_s", bufs=1, space="PSUM"))
        pp_c = ctx.enter_context(tc.tile_pool(name="pp_c", bufs=1, space="PSUM"))

        def load_tp_chunk(uid, b, c):
            """DMA-cast one key chunk to bf16 and emit its transpose
            matmuls; keyT lands in fp8 pair tiles.  Returns (knat3, ktp)."""
            knat = kpool.tile([128, 4 * kd], bf16, name=f"knat{uid}",
                              tag="knat")
            knat3 = knat[:].rearrange("p (t k) -> p t k", k=kd)
            nc.gpsimd.dma_start(
                knat3,
                key_d[b, c * SC:(c + 1) * SC, :]
                .rearrange("(t p) k -> p t k", p=128))
            # keyT tiles via normal-mode PE matmuls out = block.T @ I
            # (contraction over the s partition dim).
            ktp = []
            for j in range(nkp):
                ktj = ktpool.tile([128, 2, SC], f8, name=f"kt{uid}j{j}",
                                  tag=f"kt{j}")
                for half in range(2):
                    t = 2 * j + half
                    pst = pp_t.tile([128, SC], f32,
                                    name=f"pst{uid}t{t}", tag="pst")
                    for sp in range(4):
                        nc.tensor.matmul(
                            pst[:, sp * 128:(sp + 1) * 128],
                            knat3[:, sp, t * 128:(t + 1) * 128],
                            ident_b[:], start=True, stop=True)
                    nc.vector.tensor_copy(ktj[:, half, :], pst[:])
                ktp.append(ktj)
            return knat3, ktp

        # Hoist chunk (b=0, c=0): its key DMA goes out on the SWDGE queue
        # BEFORE the weight DMAs, and its transpose matmuls lead the PE
        # program so the PE has work while the weights stream in.
        pre = None
        if not dyn_reps and reps == 1:
            pre = load_tp_chunk("pre", 0, 0)

        # ---------------- weight prep (once per core) ----------------
        with tc.tile_pool(name="wprep", bufs=1) as wprep:
            w_nat = [wprep.tile([128, kd], f32r, name=f"wnat{m}",
                                tag=f"wn{m}") for m in range(4)]

            # --- WeT pair tiles (fp8) ---
            for half in range(0, nat, 4):
                n = min(4, nat - half)
                for j in range(n):
                    nc.gpsimd.dma_start(
                        w_nat[j][:],
                        we_d[(half + j) * 128:(half + j + 1) * 128, :])
                for t in range(nkt):
                    ps = pp_t.tile([128, 512], f32,
                                   name=f"wps{t}_{half}", tag="pst")
                    for j in range(n):
                        nc.tensor.matmul(
                            ps[:, j * 128:(j + 1) * 128],
                            w_nat[j][:, t * 128:(t + 1) * 128],
                            ident_r[:], start=True, stop=True)
                    nc.vector.tensor_copy(
                        wet[t // 2][:, t % 2, half * 128:(half + n) * 128],
                        ps[:, :n * 128])

            # --- qT ---
            qn = wprep.tile([bs, qd], f32, name="qn")
            nc.sync.dma_start(qn[:], q_d)
            psq = pp_s.tile([128, nqt * bs], f32, name="psq", tag="pse")
            for t in range(nqt):
                nc.tensor.matmul(psq[:, t * bs:(t + 1) * bs],
                                 qn[:, t * 128:(t + 1) * 128],
                                 ident_f[:bs, :bs], is_transpose=True)
            qt = wprep.tile([128, nqt * bs], f32r, name="qt")
            nc.vector.tensor_copy(qt[:], psq[:])

            # --- V = q @ Wd.T, streaming Wd one q-column-tile at a time.
            psv_all = pp_s.tile([128, nat * bs], f32, name="psv_all",
                                tag="pss")
            for t in range(nqt):
                wdc = wprep.tile([128, nat * 128], f32r, name=f"wdc{t}",
                                 tag="wdc", bufs=2)
                wdc3 = wdc[:].rearrange("p (j q) -> p j q", q=128)
                nc.gpsimd.dma_start(
                    wdc3,
                    wd_d[:, t * 128:(t + 1) * 128]
                    .rearrange("(j p) q -> p j q", p=128))
                wdt_t = wprep.tile([128, ad], f32r, name=f"wdt{t}",
                                   tag="wdt", bufs=2)
                for j in range(nat):
                    ps = pp_t.tile([128, 512], f32, name=f"wdps{t}_{j}",
                                   tag="pst", bufs=2)
                    nc.tensor.matmul(ps[:, :128], wdc3[:, j, :],
                                     ident_r[:], start=True, stop=True)
                    nc.vector.tensor_copy(wdt_t[:, j * 128:(j + 1) * 128],
                                          ps[:, :128])
                for m in range(nat):
                    nc.tensor.matmul(
                        psv_all[:, m * bs:(m + 1) * bs],
                        wdt_t[:, m * 128:(m + 1) * 128],
                        qt[:, t * bs:(t + 1) * bs],
                        start=(t == 0 and m == 0),
                        stop=(t == nqt - 1 and m == nat - 1),
                        skip_group_check=True)
            for m in range(nat):
                nc.vector.tensor_copy(vbias[m][:],
                                      psv_all[:, m * bs:(m + 1) * bs])

            # --- v columns ---
            vrow = wprep.tile([1, ad], f32, name="vrow")
            nc.sync.dma_start(vrow[:], v_d)
            psvc = pp_s.tile([128, nat], f32, name="psvc", tag="pss")
            for m in range(nat):
                nc.tensor.matmul(psvc[:, m:m + 1],
                                 vrow[:, m * 128:(m + 1) * 128],
                                 one_f[:], is_transpose=True)
            nc.vector.tensor_copy(vcols[:], psvc[:])

        # ---------------- main streaming loop ----------------
        nkh = kd // 512  # context free-dim chunks

        def emit_body(rep):
            for b in range(bs):
                tagb = f"r{rep}b{b}"
                zparts = spool.tile([1, nsc], f32, name=f"zp{tagb}",
                                    tag="zparts")
                ctx_ps = [pp_c.tile([1, 512], f32, name=f"ctx{tagb}_{h}",
                                    tag=f"ctx{h}") for h in range(nkh)]

                def emit_tail(c, erow, knat3):
                    # 7. e-row -> e-columns [128, 4] (fp32 transpose-mode)
                    pse = pp_s.tile([128, 4], f32, name=f"pse{tagb}c{c}",
                                    tag="pse")
                    for sp in range(4):
                        nc.tensor.matmul(pse[:, sp:sp + 1],
                                         erow[:, sp * 128:(sp + 1) * 128],
                                         one_f[:], is_transpose=True)
                    ecol = spool.tile([128, 4], bf16, name=f"ec{tagb}c{c}",
                                      tag="ecol")
                    nc.vector.tensor_copy(ecol[:], pse[:])

                    # 8. context accumulation (contract over s), bf16
                    for sp in range(4):
                        for h in range(nkh):
                            nc.tensor.matmul(
                                ctx_ps[h][:], ecol[:, sp:sp + 1],
                                knat3[:, sp, h * 512:(h + 1) * 512],
                                start=(c == 0 and sp == 0),
                                stop=(c == nsc - 1 and sp == 3))

                pending = None
                for c in range(nsc):
                    # 1+2. key chunk load + keyT transposes
                    if pre is not None and rep == 0 and b == 0 and c == 0:
                        knat3, ktp = pre
                    else:
                        knat3, ktp = load_tp_chunk(f"{tagb}c{c}", b, c)

                    # 3+4. U^T a-tiles via DoubleRow fp8, tanh(U+V) on ACT
                    ths = []
                    for m in range(nat):
                        psu = pp_u.tile([128, SC], f32,
                                        name=f"psu{tagb}c{c}m{m}", tag="psu")
                        for j in range(nkp):
                            nc.tensor.matmul(
                                psu[:],
                                wet[j][:, :, m * 128:(m + 1) * 128],
                                ktp[j][:],
                                start=(j == 0), stop=(j == nkp - 1),
                                perf_mode=DR)
                        th = thpool.tile([128, SC], bf16,
                                         name=f"th{tagb}c{c}m{m}", tag=f"th{m}")
                        nc.scalar.activation(th[:], psu[:], AF.Tanh,
                                             bias=vbias[m][:, b:b + 1])
                        ths.append(th)

                    # 5. score row (bf16)
                    pss = pp_s.tile([1, SC], f32, name=f"pss{tagb}c{c}",
                                    tag="pss")
                    for m in range(nat):
                        nc.tensor.matmul(pss[:], vcols[:, m:m + 1], ths[m][:],
                                         start=(m == 0), stop=(m == nat - 1))

                    # 6. e = exp(score); chunk sum via accum_out
                    erow = spool.tile([1, SC], f32, name=f"erow{tagb}c{c}",
                                      tag="erow")
                    nc.scalar.activation(erow[:], pss[:], AF.Exp,
                                         accum_out=zparts[:, c:c + 1])

                    # 7+8 for the PREVIOUS chunk (deferred one chunk)
                    if pending is not None:
                        emit_tail(*pending)
                    pending = (c, erow, knat3)
                emit_tail(*pending)

                # batch epilogue: normalize and store
                z = spool.tile([1, 1], f32, name=f"z{tagb}", tag="z")
                nc.vector.reduce_sum(z[:], zparts[:], axis=mybir.AxisListType.X)
                rz = spool.tile([1, 1], f32, name=f"rz{tagb}", tag="rz")
                nc.vector.reciprocal(rz[:], z[:])
                cout = spool.tile([1, kd], f32, name=f"cout{tagb}", tag="cout")
                for h in range(nkh):
                    nc.vector.tensor_scalar_mul(cout[:, h * 512:(h + 1) * 512],
                                                ctx_ps[h][:], rz[:])
                nc.sync.dma_start(out_d[b:b + 1, :], cout[:])

        if dyn_reps:
            with tc.For_i(0, dyn_reps, 1):
                emit_body(0)
        else:
            for rep in range(reps):
                emit_body(rep)
    return nc


_CACHE = {}


def _get_compiled(cfg):
    if cfg not in _CACHE:
        nc = bacc.Bacc("TRN2", target_bir_lowering=False, debug=False)
        build_kernel(nc, *cfg)
        nc.compile()
        _CACHE[cfg] = nc
    return _CACHE[cfg]


def kernel(**inputs):
    from concourse.bass_utils import run_bass_kernel_spmd

    key = np.asarray(inputs["key"], dtype=np.float32)
    q = np.asarray(inputs["q"], dtype=np.float32)
    we = np.asarray(inputs["W_encoder"], dtype=np.float32)
    wd = np.asarray(inputs["W_decoder"], dtype=np.float32)
    v = np.asarray(inputs["v"], dtype=np.float32)

    nc = _get_compiled((BS, S, KD, QD, AD, 1))
    in_maps = []
    for cidx in range(N_CORES):
        sl = slice(cidx * BS, (cidx + 1) * BS)
        in_maps.append({
            "key": key[sl], "q": q[sl],
            "W_encoder": we, "W_decoder": wd, "v": v,
        })
    res = run_bass_kernel_spmd(nc, in_maps, list(range(N_CORES))).results
    return np.concatenate([r["out"] for r in res], axis=0)


if __name__ == "__main__":
    pass


# revision 6
# speedup vs baseline: 1.8478x; 1.2148x over previous
"""Bahdanau additive attention Trainium2 Bass kernel (fp8 DoubleRow U).

Reference (per batch b):
    U = key @ W_encoder.T                  # [S, A]
    V = q @ W_decoder.T                    # [A]
    score = tanh(U + V) @ v[0]             # [S]
    w = softmax(score)                     # [S]
    context = w @ key                      # [KD]

Sharding: data-parallel over batch across 8 NeuronCores (4 batches/core),
weights replicated.

Precision plan (validated numerically against the fp32 reference on the
actual seed-0 inputs, rel-err ~1.1e-2 vs 2e-2 budget):
  - The dominant U matmul runs in fp8e4 (e4m3) with DoubleRow perf mode:
    each PE instruction contracts TWO 128-deep k-tiles (0.5 cyc/row).
  - key is DMA-cast fp32->bf16 into SBUF (native [s,k] layout); the
    context matmul and the PE transposes read bf16 (context in fp8 would
    blow the error budget).
  - keyT tiles are produced by normal-mode PE matmuls against a bf16
    identity; the PSUM->SBUF copy casts to fp8 for the U matmul.
  - tanh(U+V) -> bf16, score matmul bf16, softmax/context f32 epilogue.

Per-core pipeline, per batch, per s-chunk of 512:
  1. SWDGE DMA-cast key chunk fp32->bf16 into SBUF (native [s,k] layout).
  2. keyT pair-tiles [k=128, 2, s=512] fp8 via PE identity matmuls + DVE
     PSUM->SBUF fp8 cast.
  3. U^T tiles [a=128, s=512] = 4 DoubleRow fp8 matmuls (2 k-tiles each)
     accumulated in PSUM.
  4. ACT: tanh(U^T + V[a]) from PSUM (V as per-partition bias), bf16 out.
  5. score [1, 512] = v.T @ tanh-tiles accumulated over a-tiles (bf16).
  6. ACT: e = exp(score) (|score| small, no max subtraction), accum_out
     gives the chunk's sum(e).
  7. PE-transpose e-row into an e-column tile [128, 4] (bf16).
  8. context PSUM [1, 1024] += e-col.T @ key-native (bf16), accumulated
     across all chunks.
Batch epilogue: Z = sum of chunk sums, context * (1/Z) on DVE, DMA out.
"""
import sys
sys.path.insert(0, "/opt/trn_rl_repo")

from contextlib import ExitStack

import numpy as np

import concourse.bass as bass
import concourse.tile as tile
from concourse import bacc, masks, mybir

dt = mybir.dt
AF = mybir.ActivationFunctionType
DR = mybir.MatmulPerfMode.DoubleRow

# Full problem shape
B, S, KD, QD, AD = 32, 2048, 1024, 1024, 1024
N_CORES = 8
BS = B // N_CORES          # batches per core
SC = 512                   # s-chunk (columns per U matmul)


def build_kernel(nc, bs=BS, s=S, kd=KD, qd=QD, ad=AD, reps=1, dyn_reps=0):
    """Emit the per-core kernel into `nc` (a bacc.Bacc)."""
    f32, f32r, bf16, f8 = dt.float32, dt.float32r, dt.bfloat16, dt.float8e4
    nsc = s // SC            # s-chunks per batch
    nkt = kd // 128          # k-tiles
    nkp = nkt // 2           # k-tile pairs (DoubleRow)
    nat = ad // 128          # a-tiles
    nqt = qd // 128          # q-tiles
    assert s % SC == 0 and kd % 256 == 0 and ad % 128 == 0 and qd % 128 == 0

    key_d = nc.dram_tensor("key", [bs, s, kd], f32, kind="ExternalInput").ap()
    q_d = nc.dram_tensor("q", [bs, qd], f32, kind="ExternalInput").ap()
    we_d = nc.dram_tensor("W_encoder", [ad, kd], f32, kind="ExternalInput").ap()
    wd_d = nc.dram_tensor("W_decoder", [ad, qd], f32, kind="ExternalInput").ap()
    v_d = nc.dram_tensor("v", [1, ad], f32, kind="ExternalInput").ap()
    out_d = nc.dram_tensor("out", [bs, kd], f32, kind="ExternalOutput").ap()

    with tile.TileContext(nc) as tc, ExitStack() as ctx:
        const = ctx.enter_context(tc.tile_pool(name="const", bufs=1))

        ident_f = const.tile([128, 128], f32, name="ident_f")
        masks.make_identity(nc, ident_f[:])
        ident_r = const.tile([128, 128], f32r, name="ident_r")
        nc.vector.tensor_copy(ident_r[:], ident_f[:])
        ident_b = const.tile([128, 128], bf16, name="ident_b")
        nc.vector.tensor_copy(ident_b[:], ident_f[:])
        one_f = const.tile([1, 1], f32, name="one_f")
        nc.gpsimd.memset(one_f[:], 1.0)

        # WeT pair tiles [k=128, 2, a=ad] fp8, one per k-tile pair.
        wet = [const.tile([128, 2, ad], f8, name=f"wet{t}") for t in range(nkp)]
        # V bias [a-tile][128, bs] fp32 and v columns [128, nat] bf16.
        vbias = [const.tile([128, bs], f32, name=f"vbias{m}") for m in range(nat)]
        vcols = const.tile([128, nat], bf16, name="vcols")

        # ---------------- pools ----------------
        kpool = ctx.enter_context(tc.tile_pool(name="knat", bufs=3))
        ktpool = ctx.enter_context(tc.tile_pool(name="keyT", bufs=2))
        thpool = ctx.enter_context(tc.tile_pool(name="tanh", bufs=2))
        spool = ctx.enter_context(tc.tile_pool(name="small", bufs=2))
        pp_t = ctx.enter_context(tc.tile_pool(name="pp_t", bufs=2, space="PSUM"))
        pp_u = ctx.enter_context(tc.tile_pool(name="pp_u", bufs=2, space="PSUM"))
        pp_s = ctx.enter_context(tc.tile_pool(name="pp_s", bufs=1, space="PSUM"))
        pp_c = ctx.enter_context(tc.tile_pool(name="pp_c", bufs=1, space="PSUM"))

        def load_tp_chunk(uid, b, c):
            """DMA-cast one key chunk to bf16 and emit its transpose
            matmuls; keyT lands in fp8 pair tiles.  Returns (knat3, ktp)."""
            knat = kpool.tile([128, 4 * kd], bf16, name=f"knat{uid}",
                              tag="knat")
            knat3 = knat[:].rearrange("p (t k) -> p t k", k=kd)
            nc.gpsimd.dma_start(
                knat3,
                key_d[b, c * SC:(c + 1) * SC, :]
                .rearrange("(t p) k -> p t k", p=128))
            # keyT tiles via normal-mode PE matmuls out = block.T @ I
            # (contraction over the s partition dim).
            ktp = []
            for j in range(nkp):
                ktj = ktpool.tile([128, 2, SC], f8, name=f"kt{uid}j{j}",
                                  tag=f"kt{j}")
                for half in range(2):
                    t = 2 * j + half
                    pst = pp_t.tile([128, SC], f32,
                                    name=f"pst{uid}t{t}", tag="pst")
                    for sp in range(4):
                        nc.tensor.matmul(
                            pst[:, sp * 128:(sp + 1) * 128],
                            knat3[:, sp, t * 128:(t + 1) * 128],
                            ident_b[:], start=True, stop=True)
                    nc.vector.tensor_copy(ktj[:, half, :], pst[:])
                ktp.append(ktj)
            return knat3, ktp

        # Hoist chunk (b=0, c=0): its key DMA goes out on the SWDGE queue
        # BEFORE the weight DMAs, and its transpose matmuls lead the PE
        # program so the PE has work while the weights stream in.
        pre = None
        if not dyn_reps and reps == 1:
            pre = load_tp_chunk("pre", 0, 0)

        # ---------------- weight prep (once per core) ----------------
        with tc.tile_pool(name="wprep", bufs=1) as wprep:
            w_nat = [wprep.tile([128, kd], f32r, name=f"wnat{m}",
                                tag=f"wn{m}") for m in range(4)]

            # --- WeT pair tiles (fp8) ---
            for half in range(0, nat, 4):
                n = min(4, nat - half)
                for j in range(n):
                    nc.gpsimd.dma_start(
                        w_nat[j][:],
                        we_d[(half + j) * 128:(half + j + 1) * 128, :])
                for t in range(nkt):
                    ps = pp_t.tile([128, 512], f32,
                                   name=f"wps{t}_{half}", tag="pst")
                    for j in range(n):
                        nc.tensor.matmul(
                            ps[:, j * 128:(j + 1) * 128],
                            w_nat[j][:, t * 128:(t + 1) * 128],
                            ident_r[:], start=True, stop=True)
                    nc.vector.tensor_copy(
                        wet[t // 2][:, t % 2, half * 128:(half + n) * 128],
                        ps[:, :n * 128])

            # --- qT ---
            qn = wprep.tile([bs, qd], f32, name="qn")
            nc.sync.dma_start(qn[:], q_d)
            psq = pp_s.tile([128, nqt * bs], f32, name="psq", tag="pse")
            for t in range(nqt):
                nc.tensor.matmul(psq[:, t * bs:(t + 1) * bs],
                                 qn[:, t * 128:(t + 1) * 128],
                                 ident_f[:bs, :bs], is_transpose=True)
            qt = wprep.tile([128, nqt * bs], f32r, name="qt")
            nc.vector.tensor_copy(qt[:], psq[:])

            # --- V = q @ Wd.T, streaming Wd one q-column-tile at a time.
            psv_all = pp_s.tile([128, nat * bs], f32, name="psv_all",
                                tag="pss")
            for t in range(nqt):
                wdc = wprep.tile([128, nat * 128], f32r, name=f"wdc{t}",
                                 tag="wdc", bufs=2)
                wdc3 = wdc[:].rearrange("p (j q) -> p j q", q=128)
                nc.gpsimd.dma_start(
                    wdc3,
                    wd_d[:, t * 128:(t + 1) * 128]
                    .rearrange("(j p) q -> p j q", p=128))
                wdt_t = wprep.tile([128, ad], f32r, name=f"wdt{t}",
                                   tag="wdt", bufs=2)
                for j in range(nat):
                    ps = pp_t.tile([128, 512], f32, name=f"wdps{t}_{j}",
                                   tag="pst", bufs=2)
                    nc.tensor.matmul(ps[:, :128], wdc3[:, j, :],
                                     ident_r[:], start=True, stop=True)
                    nc.vector.tensor_copy(wdt_t[:, j * 128:(j + 1) * 128],
                                          ps[:, :128])
                for m in range(nat):
                    nc.tensor.matmul(
                        psv_all[:, m * bs:(m + 1) * bs],
                        wdt_t[:, m * 128:(m + 1) * 128],
                        qt[:, t * bs:(t + 1) * bs],
                        start=(t == 0 and m == 0),
                        stop=(t == nqt - 1 and m == nat - 1),
                        skip_group_check=True)
            for m in range(nat):
                nc.vector.tensor_copy(vbias[m][:],
                                      psv_all[:, m * bs:(m + 1) * bs])

            # --- v columns ---
            vrow = wprep.tile([1, ad], f32, name="vrow")
            nc.sync.dma_start(vrow[:], v_d)
            psvc = pp_s.tile([128, nat], f32, name="psvc", tag="pss")
            for m in range(nat):
                nc.tensor.matmul(psvc[:, m:m + 1],
                                 vrow[:, m * 128:(m + 1) * 128],
                                 one_f[:], is_transpose=True)
            nc.vector.tensor_copy(vcols[:], psvc[:])

        # ---------------- main streaming loop ----------------
        nkh = kd // 512  # context free-dim chunks

        def emit_body(rep):
            for b in range(bs):
                tagb = f"r{rep}b{b}"
                zparts = spool.tile([1, nsc], f32, name=f"zp{tagb}",
                                    tag="zparts")
                ctx_ps = [pp_c.tile([1, 512], f32, name=f"ctx{tagb}_{h}",
                                    tag=f"ctx{h}") for h in range(nkh)]

                def emit_tail(c, erow, knat3):
                    # 7. e-row -> e-columns [128, 4] (fp32 transpose-mode)
                    pse = pp_s.tile([128, 4], f32, name=f"pse{tagb}c{c}",
                                    tag="pse")
                    for sp in range(4):
                        nc.tensor.matmul(pse[:, sp:sp + 1],
                                         erow[:, sp * 128:(sp + 1) * 128],
                                         one_f[:], is_transpose=True)
                    ecol = spool.tile([128, 4], bf16, name=f"ec{tagb}c{c}",
                                      tag="ecol")
                    nc.vector.tensor_copy(ecol[:], pse[:])

                    # 8. context accumulation (contract over s), bf16
                    for sp in range(4):
                        for h in range(nkh):
                            nc.tensor.matmul(
                                ctx_ps[h][:], ecol[:, sp:sp + 1],
                                knat3[:, sp, h * 512:(h + 1) * 512],
                                start=(c == 0 and sp == 0),
                                stop=(c == nsc - 1 and sp == 3))

                def emit_score(c, ths):
                    # 5. score row (bf16)
                    pss = pp_s.tile([1, SC], f32, name=f"pss{tagb}c{c}",
                                    tag="pss")
                    for m in range(nat):
                        nc.tensor.matmul(pss[:], vcols[:, m:m + 1], ths[m][:],
                                         start=(m == 0), stop=(m == nat - 1))

                    # 6. e = exp(score); chunk sum via accum_out
                    erow = spool.tile([1, SC], f32, name=f"erow{tagb}c{c}",
                                      tag="erow")
                    nc.scalar.activation(erow[:], pss[:], AF.Exp,
                                         accum_out=zparts[:, c:c + 1])
                    return erow

                # Two-deep software pipeline on the PE program: per chunk c
                # emit [transposes(c), score(c-1), tail(c-2), U(c)].  score
                # never waits on the same chunk's tanh, the tail's e-columns
                # are long since ready, and both fill the PE while the DVE
                # finishes the kt fp8 copies that U(c) depends on.
                pend_score = None   # (c, ths, knat3)
                pend_tail = None    # (c, erow, knat3)
                for c in range(nsc):
                    # 1+2. key chunk load + keyT transposes
                    if pre is not None and rep == 0 and b == 0 and c == 0:
                        knat3, ktp = pre
                    else:
                        knat3, ktp = load_tp_chunk(f"{tagb}c{c}", b, c)

                    # score(c-1) + tail(c-2) slot in between the transposes
                    # and U(c) on the PE
                    new_tail = None
                    if pend_score is not None:
                        sc, sths, sknat3 = pend_score
                        erow = emit_score(sc, sths)
                        if pend_tail is not None:
                            emit_tail(*pend_tail)
                        new_tail = (sc, erow, sknat3)

                    # 3+4. U^T a-tiles via DoubleRow fp8, tanh(U+V) on ACT
                    ths = []
                    for m in range(nat):
                        psu = pp_u.tile([128, SC], f32,
                                        name=f"psu{tagb}c{c}m{m}", tag="psu")
                        for j in range(nkp):
                            nc.tensor.matmul(
                                psu[:],
                                wet[j][:, :, m * 128:(m + 1) * 128],
                                ktp[j][:],
                                start=(j == 0), stop=(j == nkp - 1),
                                perf_mode=DR)
                        th = thpool.tile([128, SC], bf16,
                                         name=f"th{tagb}c{c}m{m}", tag=f"th{m}")
                        nc.scalar.activation(th[:], psu[:], AF.Tanh,
                                             bias=vbias[m][:, b:b + 1])
                        ths.append(th)

                    if new_tail is not None:
                        pend_tail = new_tail
                    pend_score = (c, ths, knat3)

                # drain the pipeline
                sc, sths, sknat3 = pend_score
                erow = emit_score(sc, sths)
                if pend_tail is not None:
                    emit_tail(*pend_tail)
                emit_tail(sc, erow, sknat3)

                # batch epilogue: normalize and store
                z = spool.tile([1, 1], f32, name=f"z{tagb}", tag="z")
                nc.vector.reduce_sum(z[:], zparts[:], axis=mybir.AxisListType.X)
                rz = spool.tile([1, 1], f32, name=f"rz{tagb}", tag="rz")
                nc.vector.reciprocal(rz[:], z[:])
                cout = spool.tile([1, kd], f32, name=f"cout{tagb}", tag="cout")
                for h in range(nkh):
                    nc.vector.tensor_scalar_mul(cout[:, h * 512:(h + 1) * 512],
                                                ctx_ps[h][:], rz[:])
                nc.sync.dma_start(out_d[b:b + 1, :], cout[:])

        if dyn_reps:
            with tc.For_i(0, dyn_reps, 1):
                emit_body(0)
        else:
            for rep in range(reps):
                emit_body(rep)
    return nc


_CACHE = {}


def _get_compiled(cfg):
    if cfg not in _CACHE:
        nc = bacc.Bacc("TRN2", target_bir_lowering=False, debug=False)
        build_kernel(nc, *cfg)
        nc.compile()
        _CACHE[cfg] = nc
    return _CACHE[cfg]


def kernel(**inputs):
    from concourse.bass_utils import run_bass_kernel_spmd

    key = np.asarray(inputs["key"], dtype=np.float32)
    q = np.asarray(inputs["q"], dtype=np.float32)
    we = np.asarray(inputs["W_encoder"], dtype=np.float32)
    wd = np.asarray(inputs["W_decoder"], dtype=np.float32)
    v = np.asarray(inputs["v"], dtype=np.float32)

    nc = _get_compiled((BS, S, KD, QD, AD, 1))
    in_maps = []
    for cidx in range(N_CORES):
        sl = slice(cidx * BS, (cidx + 1) * BS)
        in_maps.append({
            "key": key[sl], "q": q[sl],
            "W_encoder": we, "W_decoder": wd, "v": v,
        })
    res = run_bass_kernel_spmd(nc, in_maps, list(range(N_CORES))).results
    return np.concatenate([r["out"] for r in res], axis=0)


if __name__ == "__main__":
    pass


# revision 15
# speedup vs baseline: 2.1584x; 1.1681x over previous
"""Bahdanau additive attention Trainium2 Bass kernel (fp8 DoubleRow U).

Reference (per batch b):
    U = key @ W_encoder.T                  # [S, A]
    V = q @ W_decoder.T                    # [A]
    score = tanh(U + V) @ v[0]             # [S]
    w = softmax(score)                     # [S]
    context = w @ key                      # [KD]

Sharding: data-parallel over batch across 8 NeuronCores (4 batches/core),
weights replicated.

Precision plan (validated numerically against the fp32 reference on the
actual seed-0 inputs, rel-err ~1.1e-2 vs 2e-2 budget):
  - The dominant U matmul runs in fp8e4 (e4m3) with DoubleRow perf mode:
    each PE instruction contracts TWO 128-deep k-tiles (0.5 cyc/row).
  - key is DMA-cast fp32->bf16 into SBUF (native [s,k] layout); the
    context matmul and the PE transposes read bf16 (context in fp8 would
    blow the error budget).
  - keyT tiles are produced by normal-mode PE matmuls against a bf16
    identity; the PSUM->SBUF copy casts to fp8 for the U matmul.
  - tanh(U+V) -> bf16, score matmul bf16, softmax/context f32 epilogue.

Per-core pipeline, per batch, per s-chunk of 512:
  1. SWDGE DMA-cast key chunk fp32->bf16 into SBUF (native [s,k] layout).
  2. keyT pair-tiles [k=128, 2, s=512] fp8 via PE identity matmuls + DVE
     PSUM->SBUF fp8 cast.
  3. U^T tiles [a=128, s=512] = 4 DoubleRow fp8 matmuls (2 k-tiles each)
     accumulated in PSUM.
  4. ACT: tanh(U^T + V[a]) from PSUM (V as per-partition bias), bf16 out.
  5. score [1, 512] = v.T @ tanh-tiles accumulated over a-tiles (bf16).
  6. ACT: e = exp(score) (|score| small, no max subtraction), accum_out
     gives the chunk's sum(e).
  7. PE-transpose e-row into an e-column tile [128, 4] (bf16).
  8. context PSUM [1, 1024] += e-col.T @ key-native (bf16), accumulated
     across all chunks.
Batch epilogue: Z = sum of chunk sums, context * (1/Z) on DVE, DMA out.
"""
import sys
sys.path.insert(0, "/opt/trn_rl_repo")

from contextlib import ExitStack

import numpy as np

import concourse.bass as bass
import concourse.tile as tile
from concourse import bacc, masks, mybir

dt = mybir.dt
AF = mybir.ActivationFunctionType
DR = mybir.MatmulPerfMode.DoubleRow

# Full problem shape
B, S, KD, QD, AD = 32, 2048, 1024, 1024, 1024
N_CORES = 8
BS = B // N_CORES          # batches per core
SC = 512                   # s-chunk (columns per U matmul)
SCORE_FP8 = True           # fp8 DoubleRow score matmul (tanh/v in e4m3)


def build_kernel(nc, bs=BS, s=S, kd=KD, qd=QD, ad=AD, reps=1, dyn_reps=0):
    """Emit the per-core kernel into `nc` (a bacc.Bacc)."""
    f32, f32r, bf16, f8 = dt.float32, dt.float32r, dt.bfloat16, dt.float8e4
    nsc = s // SC            # s-chunks per batch
    nkt = kd // 128          # k-tiles
    nkp = nkt // 2           # k-tile pairs (DoubleRow)
    nat = ad // 128          # a-tiles
    nqt = qd // 128          # q-tiles
    assert s % SC == 0 and kd % 256 == 0 and ad % 128 == 0 and qd % 128 == 0

    key_d = nc.dram_tensor("key", [bs, s, kd], f32, kind="ExternalInput").ap()
    q_d = nc.dram_tensor("q", [bs, qd], f32, kind="ExternalInput").ap()
    we_d = nc.dram_tensor("W_encoder", [ad, kd], f32, kind="ExternalInput").ap()
    wd_d = nc.dram_tensor("W_decoder", [ad, qd], f32, kind="ExternalInput").ap()
    v_d = nc.dram_tensor("v", [1, ad], f32, kind="ExternalInput").ap()
    out_d = nc.dram_tensor("out", [bs, kd], f32, kind="ExternalOutput").ap()

    with tile.TileContext(nc) as tc, ExitStack() as ctx:
        const = ctx.enter_context(tc.tile_pool(name="const", bufs=1))

        ident_f = const.tile([128, 128], f32, name="ident_f")
        masks.make_identity(nc, ident_f[:])
        ident_r = const.tile([128, 128], f32r, name="ident_r")
        nc.vector.tensor_copy(ident_r[:], ident_f[:])
        ident_b = const.tile([128, 128], bf16, name="ident_b")
        nc.vector.tensor_copy(ident_b[:], ident_f[:])
        one_f = const.tile([1, 1], f32, name="one_f")
        nc.gpsimd.memset(one_f[:], 1.0)

        # WeT pair tiles [k=128, 2, a=ad] fp8, one per k-tile pair.
        wet = [const.tile([128, 2, ad], f8, name=f"wet{t}") for t in range(nkp)]
        # V bias [a-tile][128, bs] fp32 and v columns [128, nat, 1].
        vbias = [const.tile([128, bs], f32, name=f"vbias{m}") for m in range(nat)]
        # For the DR score the v columns live replicated 16x so the pair
        # dim has a 16-byte stride (s3_lw_dual_fp8_restrictions).
        sc_dt = f8 if SCORE_FP8 else bf16
        sc_w = 16 if SCORE_FP8 else 1
        vcols = const.tile([128, nat, sc_w], sc_dt, name="vcols")

        # ---------------- pools ----------------
        kpool = ctx.enter_context(tc.tile_pool(name="knat", bufs=3))
        ktpool = ctx.enter_context(tc.tile_pool(name="keyT", bufs=2))
        thpool = ctx.enter_context(tc.tile_pool(name="tanh", bufs=2))
        spool = ctx.enter_context(tc.tile_pool(name="small", bufs=2))
        pp_t = ctx.enter_context(tc.tile_pool(name="pp_t", bufs=2, space="PSUM"))
        pp_u = ctx.enter_context(tc.tile_pool(name="pp_u", bufs=2, space="PSUM"))
        pp_s = ctx.enter_context(tc.tile_pool(name="pp_s", bufs=1, space="PSUM"))
        pp_c = ctx.enter_context(tc.tile_pool(name="pp_c", bufs=1, space="PSUM"))

        def load_tp_chunk(uid, b, c):
            """DMA-cast one key chunk to bf16 and emit its transpose
            matmuls; keyT lands in fp8 pair tiles.  Returns (knat3, ktp)."""
            knat = kpool.tile([128, 4 * kd], bf16, name=f"knat{uid}",
                              tag="knat")
            knat3 = knat[:].rearrange("p (t k) -> p t k", k=kd)
            nc.gpsimd.dma_start(
                knat3,
                key_d[b, c * SC:(c + 1) * SC, :]
                .rearrange("(t p) k -> p t k", p=128))
            # keyT tiles via normal-mode PE matmuls out = block.T @ I
            # (contraction over the s partition dim).
            ktp = []
            for j in range(nkp):
                ktj = ktpool.tile([128, 2, SC], f8, name=f"kt{uid}j{j}",
                                  tag=f"kt{j}")
                for half in range(2):
                    t = 2 * j + half
                    pst = pp_t.tile([128, SC], f32,
                                    name=f"pst{uid}t{t}", tag="pst")
                    for sp in range(4):
                        nc.tensor.matmul(
                            pst[:, sp * 128:(sp + 1) * 128],
                            knat3[:, sp, t * 128:(t + 1) * 128],
                            ident_b[:], start=True, stop=True)
                    nc.vector.tensor_copy(ktj[:, half, :], pst[:])
                ktp.append(ktj)
            return knat3, ktp

        # Hoist chunk (b=0, c=0): its key DMA goes out on the SWDGE queue
        # BEFORE the weight DMAs, and its transpose matmuls lead the PE
        # program so the PE has work while the weights stream in.
        pre = None
        if not dyn_reps and reps == 1:
            pre = load_tp_chunk("pre", 0, 0)

        # ---------------- weight prep (once per core) ----------------
        with tc.tile_pool(name="wprep", bufs=1) as wprep:
            w_nat = [wprep.tile([128, kd], f32r, name=f"wnat{m}",
                                tag=f"wn{m}") for m in range(4)]

            # --- WeT pair tiles (fp8) ---
            for half in range(0, nat, 4):
                n = min(4, nat - half)
                for j in range(n):
                    nc.gpsimd.dma_start(
                        w_nat[j][:],
                        we_d[(half + j) * 128:(half + j + 1) * 128, :])
                for t in range(nkt):
                    ps = pp_t.tile([128, 512], f32,
                                   name=f"wps{t}_{half}", tag="pst")
                    for j in range(n):
                        nc.tensor.matmul(
                            ps[:, j * 128:(j + 1) * 128],
                            w_nat[j][:, t * 128:(t + 1) * 128],
                            ident_r[:], start=True, stop=True)
                    nc.vector.tensor_copy(
                        wet[t // 2][:, t % 2, half * 128:(half + n) * 128],
                        ps[:, :n * 128])

            # --- qT ---
            qn = wprep.tile([bs, qd], f32, name="qn")
            nc.sync.dma_start(qn[:], q_d)
            psq = pp_s.tile([128, nqt * bs], f32, name="psq", tag="pse")
            for t in range(nqt):
                nc.tensor.matmul(psq[:, t * bs:(t + 1) * bs],
                                 qn[:, t * 128:(t + 1) * 128],
                                 ident_f[:bs, :bs], is_transpose=True)
            qt = wprep.tile([128, nqt * bs], f32r, name="qt")
            nc.vector.tensor_copy(qt[:], psq[:])

            # --- V = q @ Wd.T, streaming Wd one q-column-tile at a time.
            psv_all = pp_s.tile([128, nat * bs], f32, name="psv_all",
                                tag="pss")
            for t in range(nqt):
                wdc = wprep.tile([128, nat * 128], f32r, name=f"wdc{t}",
                                 tag="wdc", bufs=2)
                wdc3 = wdc[:].rearrange("p (j q) -> p j q", q=128)
                nc.gpsimd.dma_start(
                    wdc3,
                    wd_d[:, t * 128:(t + 1) * 128]
                    .rearrange("(j p) q -> p j q", p=128))
                wdt_t = wprep.tile([128, ad], f32r, name=f"wdt{t}",
                                   tag="wdt", bufs=2)
                for j in range(nat):
                    ps = pp_t.tile([128, 512], f32, name=f"wdps{t}_{j}",
                                   tag="pst", bufs=2)
                    nc.tensor.matmul(ps[:, :128], wdc3[:, j, :],
                                     ident_r[:], start=True, stop=True)
                    nc.vector.tensor_copy(wdt_t[:, j * 128:(j + 1) * 128],
                                          ps[:, :128])
                for m in range(nat):
                    nc.tensor.matmul(
                        psv_all[:, m * bs:(m + 1) * bs],
                        wdt_t[:, m * 128:(m + 1) * 128],
                        qt[:, t * bs:(t + 1) * bs],
                        start=(t == 0 and m == 0),
                        stop=(t == nqt - 1 and m == nat - 1),
                        skip_group_check=True)
            for m in range(nat):
                nc.vector.tensor_copy(vbias[m][:],
                                      psv_all[:, m * bs:(m + 1) * bs])

            # --- v columns ---
            vrow = wprep.tile([1, ad], f32, name="vrow")
            nc.sync.dma_start(vrow[:], v_d)
            psvc = pp_s.tile([128, nat], f32, name="psvc", tag="pss")
            for m in range(nat):
                nc.tensor.matmul(psvc[:, m:m + 1],
                                 vrow[:, m * 128:(m + 1) * 128],
                                 one_f[:], is_transpose=True)
            for j in range(sc_w):
                nc.vector.tensor_copy(vcols[:, :, j], psvc[:])

        # ---------------- main streaming loop ----------------
        nkh = kd // 512  # context free-dim chunks

        def emit_body(rep):
            for b in range(bs):
                tagb = f"r{rep}b{b}"
                zparts = spool.tile([1, nsc], f32, name=f"zp{tagb}",
                                    tag="zparts")
                ctx_ps = [pp_c.tile([1, 512], f32, name=f"ctx{tagb}_{h}",
                                    tag=f"ctx{h}") for h in range(nkh)]

                def emit_tail(c, erow, knat3):
                    # 7. e-row -> e-columns [128, 4] (fp32 transpose-mode)
                    pse = pp_s.tile([128, 4], f32, name=f"pse{tagb}c{c}",
                                    tag="pse")
                    for sp in range(4):
                        nc.tensor.matmul(pse[:, sp:sp + 1],
                                         erow[:, sp * 128:(sp + 1) * 128],
                                         one_f[:], is_transpose=True)
                    ecol = spool.tile([128, 4], bf16, name=f"ec{tagb}c{c}",
                                      tag="ecol")
                    nc.vector.tensor_copy(ecol[:], pse[:])

                    # 8. context accumulation (contract over s), bf16
                    for sp in range(4):
                        for h in range(nkh):
                            nc.tensor.matmul(
                                ctx_ps[h][:], ecol[:, sp:sp + 1],
                                knat3[:, sp, h * 512:(h + 1) * 512],
                                start=(c == 0 and sp == 0),
                                stop=(c == nsc - 1 and sp == 3))

                def emit_score(c, ths):
                    # 5. score row (fp8 DoubleRow over a-tile pairs, or bf16)
                    pss = pp_s.tile([sc_w, SC], f32, name=f"pss{tagb}c{c}",
                                    tag="pss")
                    if SCORE_FP8:
                        # 16 replicated score rows; row 0 is consumed
                        for p in range(nat // 2):
                            nc.tensor.matmul(pss[:], vcols[:, 2 * p:2 * p + 2, :],
                                             ths[p][:],
                                             start=(p == 0),
                                             stop=(p == nat // 2 - 1),
                                             perf_mode=DR)
                    else:
                        for m in range(nat):
                            nc.tensor.matmul(pss[:], vcols[:, m, :], ths[m][:],
                                             start=(m == 0), stop=(m == nat - 1))

                    # 6. e = exp(score); chunk sum via accum_out
                    erow = spool.tile([1, SC], f32, name=f"erow{tagb}c{c}",
                                      tag="erow")
                    nc.scalar.activation(erow[:], pss[0:1, :], AF.Exp,
                                         accum_out=zparts[:, c:c + 1])
                    return erow

                # Two-deep software pipeline on the PE program: per chunk c
                # emit [transposes(c), score(c-1), tail(c-2), U(c)].  score
                # never waits on the same chunk's tanh, the tail's e-columns
                # are long since ready, and both fill the PE while the DVE
                # finishes the kt fp8 copies that U(c) depends on.
                pend_score = None   # (c, ths, knat3)
                pend_tail = None    # (c, erow, knat3)
                for c in range(nsc):
                    # 1+2. key chunk load + keyT transposes
                    if pre is not None and rep == 0 and b == 0 and c == 0:
                        knat3, ktp = pre
                    else:
                        knat3, ktp = load_tp_chunk(f"{tagb}c{c}", b, c)

                    # score(c-1) + tail(c-2) slot in between the transposes
                    # and U(c) on the PE
                    new_tail = None
                    if pend_score is not None:
                        sc, sths, sknat3 = pend_score
                        erow = emit_score(sc, sths)
                        if pend_tail is not None:
                            emit_tail(*pend_tail)
                        new_tail = (sc, erow, sknat3)

                    # 3+4. U^T a-tiles via DoubleRow fp8, tanh(U+V) on ACT
                    ths = []
                    th_pair = None
                    for m in range(nat):
                        psu = pp_u.tile([128, SC], f32,
                                        name=f"psu{tagb}c{c}m{m}", tag="psu")
                        for j in range(nkp):
                            nc.tensor.matmul(
                                psu[:],
                                wet[j][:, :, m * 128:(m + 1) * 128],
                                ktp[j][:],
                                start=(j == 0), stop=(j == nkp - 1),
                                perf_mode=DR)
                        if SCORE_FP8:
                            # tanh lands in fp8 pair tiles for the DR score
                            if m % 2 == 0:
                                th_pair = thpool.tile(
                                    [128, 2, SC], f8,
                                    name=f"th{tagb}c{c}p{m // 2}",
                                    tag=f"th{m // 2}")
                                ths.append(th_pair)
                            nc.scalar.activation(th_pair[:, m % 2, :], psu[:],
                                                 AF.Tanh,
                                                 bias=vbias[m][:, b:b + 1])
                        else:
                            th = thpool.tile([128, SC], bf16,
                                             name=f"th{tagb}c{c}m{m}",
                                             tag=f"th{m}")
                            nc.scalar.activation(th[:], psu[:], AF.Tanh,
                                                 bias=vbias[m][:, b:b + 1])
                            ths.append(th)

                    if new_tail is not None:
                        pend_tail = new_tail
                    pend_score = (c, ths, knat3)

                # drain the pipeline
                sc, sths, sknat3 = pend_score
                erow = emit_score(sc, sths)
                if pend_tail is not None:
                    emit_tail(*pend_tail)
                emit_tail(sc, erow, sknat3)

                # batch epilogue: normalize and store
                z = spool.tile([1, 1], f32, name=f"z{tagb}", tag="z")
                nc.vector.reduce_sum(z[:], zparts[:], axis=mybir.AxisListType.X)
                rz = spool.tile([1, 1], f32, name=f"rz{tagb}", tag="rz")
                nc.vector.reciprocal(rz[:], z[:])
                cout = spool.tile([1, kd], f32, name=f"cout{tagb}", tag="cout")
                for h in range(nkh):
                    nc.vector.tensor_scalar_mul(cout[:, h * 512:(h + 1) * 512],
                                                ctx_ps[h][:], rz[:])
                nc.sync.dma_start(out_d[b:b + 1, :], cout[:])

        if dyn_reps:
            with tc.For_i(0, dyn_reps, 1):
                emit_body(0)
        else:
            for rep in range(reps):
                emit_body(rep)
    return nc


_CACHE = {}


def _get_compiled(cfg):
    if cfg not in _CACHE:
        nc = bacc.Bacc("TRN2", target_bir_lowering=False, debug=False)
        build_kernel(nc, *cfg)
        nc.compile()
        _CACHE[cfg] = nc
    return _CACHE[cfg]


def kernel(**inputs):
    from concourse.bass_utils import run_bass_kernel_spmd

    key = np.asarray(inputs["key"], dtype=np.float32)
    q = np.asarray(inputs["q"], dtype=np.float32)
    we = np.asarray(inputs["W_encoder"], dtype=np.float32)
    wd = np.asarray(inputs["W_decoder"], dtype=np.float32)
    v = np.asarray(inputs["v"], dtype=np.float32)

    nc = _get_compiled((BS, S, KD, QD, AD, 1))
    in_maps = []
    for cidx in range(N_CORES):
        sl = slice(cidx * BS, (cidx + 1) * BS)
        in_maps.append({
            "key": key[sl], "q": q[sl],
            "W_encoder": we, "W_decoder": wd, "v": v,
        })
    res = run_bass_kernel_spmd(nc, in_maps, list(range(N_CORES))).results
    return np.concatenate([r["out"] for r in res], axis=0)


if __name__ == "__main__":
    pass
